# revision 2
# baseline (speedup 1.0000x reference)
"""Trainium2 Bass kernel for DirectionalHMAGAT message passing, v2.

Edge pass fused with node pass (no DRAM round-trip):
- Core c owns dst nodes [c*6272, (c+1)*6272) = 49 groups x 128 dst nodes.
- Edges packed per group into 128-slot sub-batches (slot i -> partition
  i%128, sub-batch i//128). Host ships edge-ordered copies of x (xsrcT for
  the PE lhsT, xsrc/xdst row-major), per-edge weights, and the dst one-hot
  matrices -- all pure index-based layout prep.
- Per sub-batch: usrc = xsrcT^T @ (W_att/scale) on PE; scores on DVE
  (quad-batched); softmax weights per quad (ACT exp, Pool lrelu/mult);
  rhs = [xsrc*ew_h | ew] on Pool; numer[n,:] += oh^T @ rhs on PE (PSUM).
- Group tail: out = (numer/denom) @ blockdiag(W_lin) + bias -> DMA out.

Work distribution: PE matmuls; ACT PSUM->SBUF bf16 copies + exp; DVE
score mult+reduce + normalize; Pool (gpsimd) lrelu/ew/rhs mult/bias add.
"""

import json

import ml_dtypes
import numpy as np

from concourse import bass, mybir
from concourse.bass_utils import run_bass_kernel_spmd
from concourse.tile import TileContext


def _legalize_sync_waits(bir: bytes) -> bytes:
    """Walrus accepts at most one sync wait per instruction; hoist extras
    onto single-wait NoOps on the same engine."""
    m = json.loads(bir)
    k = 0
    changed = False
    for fn in m["functions"]:
        for b in fn["blocks"]:
            out = []
            for inst in b["instructions"]:
                sy = inst.get("sync_info")
                waits = sy.get("on_wait") if sy else None
                if waits and len(waits) > 1:
                    changed = True
                    for w in waits[:-1]:
                        k += 1
                        out.append({
                            "debug": inst.get("debug"),
                            "engine": inst["engine"],
                            "ins": [],
                            "outs": [],
                            "name": f"I-waitfix-{k}",
                            "opcode": "NoOp",
                            "sync_info": {"on_update": [], "on_wait": [w]},
                        })
                    sy["on_wait"] = [waits[-1]]
                out.append(inst)
            b["instructions"] = out
    if not changed:
        return bir
    return json.dumps(m).encode()


if not getattr(bass.Bass, "_waitfix_patched", False):
    _orig_to_json_bytes = bass.Bass.to_json_bytes

    def _to_json_bytes_fixed(self):
        return _legalize_sync_waits(_orig_to_json_bytes(self))

    bass.Bass.to_json_bytes = _to_json_bytes_fixed
    bass.Bass._waitfix_patched = True

# Problem constants (hardcoded per harness contract)
N, F, H, C, E = 50000, 64, 4, 64, 800000
SCALE = float(np.sqrt(F))
NEG = 0.2
NCORES = 8
NPC = 6272            # nodes per core (49 * 128; 8*6272 = 50176 >= N)
NG = NPC // 128       # 49 groups of 128 dst nodes
HF = H * F            # 256
NW = HF + H           # 260 numer cols: numerator(256) + denominator(4)
PW = 132              # edge payload row: xsrc(64) | xdst(64) | w(1) | pad(3)

f32 = mybir.dt.float32
bf16 = mybir.dt.bfloat16
AT = mybir.ActivationFunctionType
OP = mybir.AluOpType


def _prep(x, edge_index, edge_weight):
    """Pack edges per (core, 128-dst-node group) into 128-slot sub-batches.
    Returns per-core input dicts plus the shared NB-per-group plan."""
    src = np.ascontiguousarray(edge_index[0]).astype(np.int64)
    dst = np.ascontiguousarray(edge_index[1]).astype(np.int64)
    w = np.ascontiguousarray(edge_weight[:, 0]).astype(np.float32)
    xbf = np.zeros((NCORES * NPC, F), ml_dtypes.bfloat16)
    xbf[:N] = x.astype(ml_dtypes.bfloat16)

    per_core = []
    for c in range(NCORES):
        lo, hi = c * NPC, (c + 1) * NPC
        m = (dst >= lo) & (dst < hi)
        s_c, d_c, w_c = src[m], dst[m], w[m]
        o = np.argsort(d_c, kind="stable")
        s_c, d_c, w_c = s_c[o], d_c[o], w_c[o]
        bounds = np.searchsorted(d_c, lo + 128 * np.arange(NG + 1))
        per_core.append((s_c, d_c, w_c, bounds))

    NB = np.zeros(NG, np.int64)
    for g in range(NG):
        ne = max(int(pc[3][g + 1] - pc[3][g]) for pc in per_core)
        NB[g] = max(1, -(-ne // 128))
    offb = np.concatenate([[0], np.cumsum(NB)])
    TOTB = int(offb[-1])

    in_maps = []
    for c in range(NCORES):
        s_c, d_c, w_c, bounds = per_core[c]
        xsrcT = np.zeros((64, TOTB * 128), ml_dtypes.bfloat16)
        epay = np.zeros((128, TOTB, PW), ml_dtypes.bfloat16)
        oh = np.zeros((128, TOTB, 128), ml_dtypes.bfloat16)
        for g in range(NG):
            st, en = int(bounds[g]), int(bounds[g + 1])
            ne = en - st
            if ne == 0:
                continue
            k = np.arange(ne)
            p, b = k % 128, k // 128 + offb[g]
            ss, dd = s_c[st:en], d_c[st:en]
            xsrcT[:, b * 128 + p] = xbf[ss].T
            epay[p, b, 0:64] = xbf[ss]
            epay[p, b, 64:128] = xbf[dd]
            epay[p, b, 128] = w_c[st:en]
            doff = (dd - (c * NPC + g * 128)).astype(np.int64)
            oh[p, b, doff] = 1.0  # lhsT layout: oh[e, b, n]
        in_maps.append({
            "xsrcT": np.ascontiguousarray(xsrcT),
            "epay": np.ascontiguousarray(epay.reshape(128, TOTB * PW)),
            "oh": np.ascontiguousarray(oh.reshape(128, TOTB * 128)),
        })
    return in_maps, tuple(int(v) for v in NB), offb


_build_cache = {}


def _build(NB):
    if NB in _build_cache:
        return _build_cache[NB]
    offb = np.concatenate([[0], np.cumsum(NB)])
    TOTB = int(offb[-1])
    NBMAX = max(NB)

    nc = bass.Bass(num_swdge_queues=4)
    xsrcT_d = nc.declare_dram_parameter("xsrcT", [64, TOTB * 128], bf16, isOutput=False)
    epay_d = nc.declare_dram_parameter("epay", [128, TOTB * PW], bf16, isOutput=False)
    oh_d = nc.declare_dram_parameter("oh", [128, TOTB * 128], bf16, isOutput=False)
    watt2_d = nc.declare_dram_parameter("watt2", [64, HF], bf16, isOutput=False)
    wbd_d = nc.declare_dram_parameter("wbd", [2, 128, HF], bf16, isOutput=False)
    biasb_d = nc.declare_dram_parameter("biasb", [128, HF], f32, isOutput=False)
    out_d = nc.declare_dram_parameter("out", [NPC, HF], f32, isOutput=True)

    with nc.allow_low_precision(reason="bf16 scores/messages by design; "
                                "softmax tolerant, gate is 2e-2"), \
         TileContext(nc) as tc:
        with tc.tile_pool(name="const", bufs=1) as cp:
            watt2_s = cp.tile([64, HF], bf16)
            nc.sync.dma_start(watt2_s[:], watt2_d[:])
            wbd_a = cp.tile([128, HF], bf16)
            nc.sync.dma_start(wbd_a[:], wbd_d[0])
            wbd_b = cp.tile([128, HF], bf16)
            nc.sync.dma_start(wbd_b[:], wbd_d[1])
            biasb = cp.tile([128, HF], f32)
            nc.sync.dma_start(biasb[:], biasb_d[:])
            eps_t = cp.tile([128, 1], f32)
            nc.gpsimd.memset(eps_t[:], 1e-16)

            with (
                tc.tile_pool(name="gp", bufs=2) as gp,
                tc.tile_pool(name="qp", bufs=3) as qp,
                tc.tile_pool(name="ups", bufs=2, space="PSUM") as ups,
                tc.tile_pool(name="nps", bufs=2, space="PSUM") as nps,
            ):
                def phaseA(g):
                    """DMAs + usrc matmuls + PSUM copies + score mult/reduce."""
                    nb = NB[g]
                    ob = int(offb[g])
                    nq = -(-nb // 4)
                    xsrcT_t = gp.tile([64, nb * 128], bf16, tag="xsrcT",
                                      padded_shape=[64, NBMAX * 128],
                                      name=f"xsrcT_g{g}")
                    nc.sync.dma_start(xsrcT_t[:], xsrcT_d[:, ob * 128:(ob + nb) * 128])
                    epay_t = gp.tile([128, nb, PW], bf16, tag="epay",
                                     padded_shape=[128, NBMAX, PW],
                                     name=f"epay_g{g}")
                    nc.sync.dma_start(epay_t[:], epay_d[:, ob * PW:(ob + nb) * PW]
                                      .rearrange("p (b q) -> p b q", q=PW))
                    oh_t = gp.tile([128, nb, 128], bf16, tag="oh",
                                   padded_shape=[128, NBMAX, 128],
                                   name=f"oh_g{g}")
                    nc.sync.dma_start(oh_t[:], oh_d[:, ob * 128:(ob + nb) * 128]
                                      .rearrange("p (b n) -> p b n", n=128))
                    score_t = gp.tile([128, nb, H], bf16, tag="score",
                                      padded_shape=[128, NBMAX, H],
                                      name=f"score_g{g}")
                    rhs_t = gp.tile([128, nb, NW], bf16, tag="rhs",
                                    padded_shape=[128, NBMAX, NW],
                                    name=f"rhs_g{g}", bufs=2)
                    numer_ps = nps.tile([128, NW], f32, tag="numer",
                                        name=f"numer_g{g}", bufs=2)
                    for q in range(nq):
                        b0 = q * 4
                        bq = min(4, nb - b0)
                        usrc_ps = ups.tile([128, 4, HF], f32, tag="usrc",
                                           name=f"usrc_g{g}q{q}")
                        for j in range(bq):
                            nc.tensor.matmul(
                                usrc_ps[:, j, :],
                                lhsT=xsrcT_t[:, (b0 + j) * 128:(b0 + j + 1) * 128],
                                rhs=watt2_s[:], start=True, stop=True)
                        usrc_sb = qp.tile([128, 4, HF], bf16, tag="usrc_sb",
                                          name=f"usrc_sb_g{g}q{q}")
                        nc.scalar.copy(usrc_sb[:, 0:bq], usrc_ps[:, 0:bq])
                        scr = qp.tile([128, 4, H, F], bf16, tag="scr",
                                      name=f"scr_g{g}q{q}")
                        nc.vector.tensor_tensor(
                            scr[:, 0:bq],
                            usrc_sb[:, 0:bq].rearrange("p b (h f) -> p b h f", h=H),
                            epay_t[:, b0:b0 + bq, 64:128]
                            .rearrange("p b (o f) -> p b o f", o=1)
                            .to_broadcast([128, bq, H, F]),
                            op=OP.mult)
                        nc.vector.tensor_reduce(
                            score_t[:, b0:b0 + bq, :], scr[:, 0:bq],
                            axis=mybir.AxisListType.X, op=OP.add)
                        slr = qp.tile([128, 4, H], bf16, tag="slr",
                                      name=f"slr_g{g}q{q}")
                        nc.vector.scalar_tensor_tensor(
                            slr[:, 0:bq], score_t[:, b0:b0 + bq, :], NEG,
                            score_t[:, b0:b0 + bq, :], op0=OP.mult, op1=OP.max)
                        e1 = qp.tile([128, 4, H], bf16, tag="e1",
                                     name=f"e1_g{g}q{q}")
                        nc.scalar.activation(e1[:, 0:bq], slr[:, 0:bq], AT.Exp)
                        nc.gpsimd.tensor_tensor(
                            rhs_t[:, b0:b0 + bq, HF:NW], e1[:, 0:bq],
                            epay_t[:, b0:b0 + bq, 128:129]
                            .to_broadcast([128, bq, H]),
                            op=OP.mult)
                        nc.gpsimd.tensor_tensor(
                            rhs_t[:, b0:b0 + bq, 0:HF]
                            .rearrange("p b (h f) -> p b h f", h=H),
                            epay_t[:, b0:b0 + bq, 0:64]
                            .rearrange("p b (o f) -> p b o f", o=1)
                            .to_broadcast([128, bq, H, F]),
                            rhs_t[:, b0:b0 + bq, HF:NW]
                            .rearrange("p b (h o) -> p b h o", o=1)
                            .to_broadcast([128, bq, H, F]),
                            op=OP.mult)
                        for j in range(bq):
                            b = b0 + j
                            nc.tensor.matmul(numer_ps[:], lhsT=oh_t[:, b, :],
                                             rhs=rhs_t[:, b, 0:NW],
                                             start=(b == 0), stop=(b == nb - 1))
                    return dict(g=g, nb=nb, nq=nq, numer_ps=numer_ps)

                def phaseB3(ctx):
                    g = ctx["g"]
                    numer_ps = ctx["numer_ps"]
                    # tail: normalize + output transform
                    dn = qp.tile([128, H], f32, tag="dn", name=f"dn_g{g}")
                    nc.scalar.add(dn[:], numer_ps[:, HF:NW], eps_t[:])
                    rcp = qp.tile([128, H], f32, tag="rcp", name=f"rcp_g{g}")
                    nc.vector.reciprocal(rcp[:], dn[:])
                    agg = qp.tile([128, HF], bf16, tag="agg", name=f"agg_g{g}")
                    nc.vector.tensor_tensor(
                        agg[:].rearrange("p (h f) -> p h f", h=H),
                        numer_ps[:, 0:HF].rearrange("p (h f) -> p h f", h=H),
                        rcp[:].rearrange("p (h o) -> p h o", o=1)
                        .to_broadcast([128, H, F]),
                        op=OP.mult)
                    aggTa = qp.tile([128, 128], bf16, tag="aggTa", name=f"aggTa_g{g}")
                    nc.sync.dma_start(aggTa[:], agg[:, 0:128], transpose=True)
                    aggTb = qp.tile([128, 128], bf16, tag="aggTb", name=f"aggTb_g{g}")
                    nc.sync.dma_start(aggTb[:], agg[:, 128:256], transpose=True)
                    out_ps = nps.tile([128, HF], f32, tag="out_ps",
                                      name=f"out_ps_g{g}", bufs=1)
                    nc.tensor.matmul(out_ps[:], lhsT=aggTa[:], rhs=wbd_a[:],
                                     start=True, stop=False)
                    nc.tensor.matmul(out_ps[:], lhsT=aggTb[:], rhs=wbd_b[:],
                                     start=False, stop=True)
                    outt = qp.tile([128, HF], f32, tag="outt", name=f"outt_g{g}")
                    nc.vector.tensor_tensor(outt[:], out_ps[:], biasb[:], op=OP.add)
                    nc.sync.dma_start(out_d[g * 128:(g + 1) * 128, :], outt[:])

                for g in range(NG):
                    ctx = phaseA(g)
                    phaseB3(ctx)
    _build_cache[NB] = nc
    return nc


_last = None  # BassKernelResults of the most recent run (for test harness)


def kernel(x, edge_index, edge_weight, W_lin, W_att, bias):
    global _last
    x = np.asarray(x, np.float32)
    in_maps, NB, offb = _prep(x, np.asarray(edge_index), np.asarray(edge_weight))
    nc = _build(NB)

    # usrc = xsrc @ (W_att/SCALE)
    watt2 = np.ascontiguousarray(
        (np.asarray(W_att, np.float32) / SCALE).astype(ml_dtypes.bfloat16))
    # block-diag W_lin, split into two 128-row halves
    wl = np.asarray(W_lin, np.float32)
    bd = np.zeros((HF, HF), np.float32)
    for h in range(H):
        bd[h * F:(h + 1) * F, h * C:(h + 1) * C] = wl[:, h * C:(h + 1) * C]
    wbd = np.ascontiguousarray(bd.reshape(2, 128, HF).astype(ml_dtypes.bfloat16))
    biasb = np.ascontiguousarray(
        np.broadcast_to(np.asarray(bias, np.float32), (128, HF)).copy())
    for m in in_maps:
        m["watt2"] = watt2
        m["wbd"] = wbd
        m["biasb"] = biasb

    _last = run_bass_kernel_spmd(nc, in_maps, list(range(NCORES)))
    res = _last.results
    out = np.concatenate([res[c]["out"] for c in range(NCORES)], axis=0)
    return np.ascontiguousarray(out[:N])
